# revision 44
# baseline (speedup 1.0000x reference)
"""Complex Conv1D (VALID, stride 1) on Trainium2 — Bass/Tile, 8-core data-parallel.

Problem (hardcoded shapes):
  x_real/x_imag: [32, 4096, 64] f32, kernel_real/imag: [9, 64, 64] f32,
  bias_real/imag: [64] f32  ->  out [32, 4088, 64, 2] f32
  out_real = conv(xr, wr) - conv(xi, wi) + br
  out_imag = conv(xr, wi) + conv(xi, wr) + bi

Mapping: complex multiply as its 2x2 real block-matrix form so each tap is ONE
full 128-contract matmul:
  X_b [128, L]   rows 0:64 = xr[b].T (channels on partitions), 64:128 = xi[b].T
  W[k] [128,128] = [[wr[k], wi[k]], [-wi[k], wr[k]]]
  psum[128, T] += W[k].T @ X_b[:, l0+k : l0+k+T]   for k = 0..8
  psum rows 0:64 = real output (filters), rows 64:128 = imag output.
Batch is sharded 4-per-core across 8 cores; weights replicated. The kernel
emits the output transposed; the host restores [B, L_out, F, 2].

Schedule notes (measured on the axon-tunneled TRN2 cores):
- Per-DMA fixed cost is ~3us and effective HBM bandwidth ~200 GB/s, far below
  the 400 GB/s spec, so the kernel minimizes DMA instructions: batches are
  packed in PAIRS on the host (x: [2, 128, 2*4096] per core) giving 2 x-loads
  + 2 stores + weights + bias = 6 DMAs per core.
- bf16 operands/outputs halve DMA bytes (rel err ~3e-3, tolerance 2e-2).
- PE floor is 288 matmuls x 512 rows @ 2.4 GHz ~= 61us; evacuation runs on
  the DVE engine so the Activation queue never blocks PSUM drains.
- Warmup matmuls on a dummy tile hold the PE pstate ramp until the first x
  chunk lands (single-shot build loads a small first chunk for fast start).
"""

import numpy as np

import concourse.bacc as bacc
import concourse.bass as bass
import concourse.mybir as mybir
from concourse.tile import TileContext
from concourse.bass_utils import run_bass_kernel_spmd

B, L, CIN, KT, F = 32, 4096, 64, 9, 64
LOUT = L - KT + 1  # 4088
NCORES = 8
BPC = B // NCORES  # batches per core
PAIR = 2
NP = BPC // PAIR  # batch-pairs per core
TL = 512  # output-tile width (one PSUM bank of fp32)
NLT = (LOUT + TL - 1) // TL  # 8

MM_DT_NAME = "bfloat16"
OUT_DT_NAME = "bfloat16"


def _build_nc(
    mm_dt,
    w_dt=None,
    out_dt=None,
    xbufs=2,
    obufs=2,
    psbufs=6,
    warmup=28,
    first_xc=TL + KT - 1,
    last_split=7 * TL - 7,
    evac="dve",  # act | dve | alt
    mode="full",  # full | pe_only (no DMA) | no_store | dma_only (no compute)
    store_q="act",  # act | pool | sp
    nxd=1,  # x-load DMAs per batch pair
    xcvt=False,  # x ships bf16, DVE upconverts to f32 (read as f32r by the PE)
    loop_repeat=None,
):
    nc = bacc.Bacc("TRN2", target_bir_lowering=False, debug=False, num_devices=NCORES)
    if w_dt is None:
        w_dt = mm_dt
    f32 = mybir.dt.float32
    bf16 = mybir.dt.bfloat16
    if out_dt is None:
        out_dt = f32
    ident = mybir.ActivationFunctionType.Identity
    x_dram_dt = bf16 if xcvt else mm_dt

    x_d = nc.dram_tensor("x", [NP, 128, PAIR * L], x_dram_dt, kind="ExternalInput")
    w_d = nc.dram_tensor("w", [128, KT * 128], w_dt, kind="ExternalInput")
    bias_d = nc.dram_tensor("bias", [128, 1], f32, kind="ExternalInput")
    out_d = nc.dram_tensor(
        "out", [NP, 128, PAIR * LOUT], out_dt, kind="ExternalOutput"
    )

    with TileContext(nc) as tc:
        with (
            tc.tile_pool(name="wpool", bufs=1) as wpool,
            tc.tile_pool(name="xrpool", bufs=xbufs) as xrpool,
            tc.tile_pool(name="xpool", bufs=xbufs) as xpool,
            tc.tile_pool(name="opool", bufs=obufs) as opool,
            tc.tile_pool(name="pspool", bufs=psbufs, space="PSUM") as pspool,
        ):
            def load_x(pi, chunks):
                """DMA x chunks for a pair; with xcvt, upconvert bf16 -> f32
                on DVE into the f32r tile the PE reads."""
                if xcvt:
                    xr = xrpool.tile([128, PAIR * L], bf16, tag="xr")
                    xt = xpool.tile([128, PAIR * L], f32, tag="xt")
                    for lo, hi in chunks:
                        nc.sync.dma_start(xr[:, lo:hi], x_d[pi, :, lo:hi])
                        nc.vector.tensor_scalar_add(
                            xt[:, lo:hi], xr[:, lo:hi], 0.0
                        )
                    return xt
                xt = xpool.tile([128, PAIR * L], mm_dt, tag="xt")
                for lo, hi in chunks:
                    nc.sync.dma_start(xt[:, lo:hi], x_d[pi, :, lo:hi])
                return xt

            wt = wpool.tile([128, KT * 128], w_dt)
            bias_t = wpool.tile([128, 1], f32)
            if mode != "pe_only" and loop_repeat is None:
                # Small first chunk so real matmuls can start ~3us in.
                xt0 = load_x(
                    0, [(0, first_xc), (first_xc, L), (L, PAIR * L)]
                )
            nc.scalar.dma_start(wt[:], w_d[:])
            nc.scalar.dma_start(bias_t[:], bias_d[:])

            if warmup:
                # bf16 regardless of mm_dt: memset can't target float32r.
                dummy = wpool.tile([128, 128], mybir.dt.bfloat16)
                nc.vector.memset(dummy[:], 0.0)
                wps = pspool.tile([128, TL], f32, tag="ps")
                for _ in range(warmup):
                    nc.tensor.matmul(
                        wps[:, :128], dummy[:], dummy[:],
                        start=True, stop=True, skip_group_check=True,
                    )

            xts = None
            if mode != "pe_only" and loop_repeat is None:
                # Preload the remaining pairs too: all x traffic lands during
                # the first pair's compute, the rest runs DMA-free.
                xts = [xt0]
                for pi in range(1, NP):
                    xts.append(load_x(pi, [(0, PAIR * L)]))

            if mode == "pe_only":
                xt_fix = load_x(0, [(0, PAIR * L)])
            if mode == "dma_only":
                ot_fix = opool.tile([128, PAIR * LOUT], out_dt, tag="ot")
                nc.vector.memset(ot_fix[:], 0.0)

            import contextlib

            loop_cm = (
                tc.For_i(0, loop_repeat, 1)
                if loop_repeat is not None
                else contextlib.nullcontext()
            )
            st_eng = {"act": nc.scalar, "pool": nc.gpsimd, "sp": nc.sync}[store_q]
            with loop_cm:
                for pi in range(NP):
                    if mode == "pe_only":
                        xt = xt_fix
                    elif loop_repeat is None:
                        xt = xts[pi]
                    else:
                        xc = (PAIR * L + nxd - 1) // nxd
                        xt = load_x(
                            pi,
                            [
                                (i * xc, min(PAIR * L, (i + 1) * xc))
                                for i in range(nxd)
                            ],
                        )
                    ot = (
                        ot_fix
                        if mode == "dma_only"
                        else opool.tile([128, PAIR * LOUT], out_dt, tag="ot")
                    )
                    if mode != "dma_only":
                        for sb in range(PAIR):
                            xb, ob = sb * L, sb * LOUT
                            for j in range(NLT):
                                l0 = j * TL
                                t = min(TL, LOUT - l0)
                                ps = pspool.tile([128, TL], f32, tag="ps")
                                for k in range(KT):
                                    rhs = xt[:, xb + l0 + k : xb + l0 + k + t]
                                    if xcvt:
                                        rhs = rhs.bitcast(mybir.dt.float32r)
                                    nc.tensor.matmul(
                                        ps[:, :t],
                                        wt[:, k * 128 : (k + 1) * 128],
                                        rhs,
                                        start=(k == 0),
                                        stop=(k == KT - 1),
                                    )
                                if evac == "dve" or (evac == "alt" and j % 2):
                                    nc.vector.tensor_scalar_add(
                                        ot[:, ob + l0 : ob + l0 + t],
                                        ps[:, :t],
                                        bias_t[:],
                                    )
                                else:
                                    nc.scalar.activation(
                                        ot[:, ob + l0 : ob + l0 + t],
                                        ps[:, :t],
                                        ident,
                                        bias=bias_t[:],
                                    )
                    if mode in ("pe_only", "no_store"):
                        continue
                    if pi == NP - 1 and loop_repeat is None:
                        # Drain tail: bulk of the last pair early via SP, a
                        # short final chunk behind the last evac.
                        cut = LOUT + last_split
                        nc.sync.dma_start(out_d[pi, :, :cut], ot[:, :cut])
                        st_eng.dma_start(out_d[pi, :, cut:], ot[:, cut:])
                    else:
                        st_eng.dma_start(out_d[pi], ot[:])

    nc.compile()
    return nc


def _pack(x_real, x_imag, kernel_real, kernel_imag, bias_real, bias_imag, np_dt,
          w_np_dt=None):
    if w_np_dt is None:
        w_np_dt = np_dt
    X = np.empty((B, 128, L), np.float32)
    X[:, :CIN] = x_real.transpose(0, 2, 1)
    X[:, CIN:] = x_imag.transpose(0, 2, 1)
    # Pack batch pairs side by side: XP[p, :, sb*L:(sb+1)*L] = X[2p+sb]
    XP = X.reshape(B // PAIR, PAIR, 128, L).transpose(0, 2, 1, 3).reshape(
        B // PAIR, 128, PAIR * L
    )
    Wk = np.empty((KT, 128, 128), np.float32)
    Wk[:, :CIN, :F] = kernel_real
    Wk[:, :CIN, F:] = kernel_imag
    Wk[:, CIN:, :F] = -kernel_imag
    Wk[:, CIN:, F:] = kernel_real
    W2 = Wk.transpose(1, 0, 2).reshape(128, KT * 128).astype(w_np_dt)
    bias2 = (
        np.concatenate([bias_real, bias_imag]).reshape(128, 1).astype(np.float32)
    )
    return (
        np.ascontiguousarray(XP.astype(np_dt)),
        np.ascontiguousarray(W2),
        bias2,
    )


def _parse_dt(name):
    name = name or MM_DT_NAME
    if name == "cvt":
        return mybir.dt.float32r, mybir.dt.float32r, mybir.dt.bfloat16
    parts = name.split(",")
    xn = parts[0]
    wn = parts[1] if len(parts) > 1 else xn
    on = parts[2] if len(parts) > 2 else OUT_DT_NAME
    return getattr(mybir.dt, xn), getattr(mybir.dt, wn), getattr(mybir.dt, on)


def _prepare(inputs, mm_dt_name=None, build_kw=None):
    build_kw = dict(build_kw or {})
    mm_dt, w_dt, out_dt = _parse_dt(mm_dt_name)
    if (mm_dt_name or MM_DT_NAME) == "cvt":
        build_kw.setdefault("xcvt", True)
    np_dt = (
        mybir.dt.np(mybir.dt.bfloat16)
        if build_kw.get("xcvt")
        else mybir.dt.np(mm_dt)
    )
    w_np_dt = mybir.dt.np(w_dt)
    args = {
        k: np.asarray(inputs[k], np.float32)
        for k in (
            "x_real", "x_imag", "kernel_real", "kernel_imag", "bias_real", "bias_imag",
        )
    }
    XP, W2, bias2 = _pack(np_dt=np_dt, w_np_dt=w_np_dt, **args)

    nc = _build_nc(mm_dt, w_dt=w_dt, out_dt=out_dt, **(build_kw or {}))
    in_maps = [
        {
            "x": np.ascontiguousarray(XP[i * NP : (i + 1) * NP]),
            "w": W2,
            "bias": bias2,
        }
        for i in range(NCORES)
    ]
    return nc, in_maps


def _gather(results):
    O = np.concatenate([r["out"] for r in results], axis=0)  # [B/2, 128, 2*LOUT]
    O = O.astype(np.float32).reshape(B // PAIR, 2, F, PAIR, LOUT)
    O = O.transpose(0, 3, 4, 2, 1).reshape(B, LOUT, F, 2)
    return np.ascontiguousarray(O, dtype=np.float32)


def _run(inputs, trace=False, mm_dt_name=None):
    nc, in_maps = _prepare(inputs, mm_dt_name)
    res = run_bass_kernel_spmd(nc, in_maps, core_ids=list(range(NCORES)), trace=trace)
    return _gather(res.results), res


def kernel(**inputs) -> np.ndarray:
    out, _ = _run(inputs, trace=False)
    return out
